# revision 44
# baseline (speedup 1.0000x reference)
"""Multi-head attention (B=2, S=2048, D=1024, H=16) on 8 Trainium2 NeuronCores.

Sharding: core = b*4 + hg  (b = batch, hg = head-group of 4 heads).

v3 — p-state-aware single-stream schedule:
  - Attention runs as 64 (sqc, skc) steps with PV/RS lagging scores by 2.
    Per-step PE order is PV/RS(t-2) | interleave | scores-p0 | scores-p1,
    which hides each exp's latency behind the NEXT step's PV + interleave
    so the psS double-buffer never stalls the PE.
  - One exp instruction per engine per step (fixed overhead ~0.3-0.4us
    per instruction dominates splitting): ACT takes pT0 (table exp),
    DVE takes pT1 (Schraudolph bf16 bit-trick; numerator and denominator
    share the approximation so softmax cancels the sawtooth).
  - X inputs arrive pre-packed [128, 16384] (c-major) so each of xq/xk/xv
    loads with 4 big DMAs — DMA issue costs ~0.7us of engine time each,
    so descriptor count matters more than ordering finesse.
  - Bias matmuls eliminated: bk drops out (softmax shift invariance),
    bv folds into a host-side constant, bq rides the q-proj drain as an
    ACT activation(Identity, bias=per-partition-column).
  - The out-projection drains straight from PSUM to DRAM via DMA (f32
    outT; host casts) — no SBUF staging, no engine drain.
  - v-proj / k-proj c1-3 / q-proj c1-3 / out-proj interleave inside the
    attention steps; two extra slots sit at each sweep boundary between
    PV(14) and PV(15).

All matmul inputs bf16 (PSUM f32). Host adds bo + bv@Wo.T and sums the
4 head-group partials per batch.
"""

import math

import numpy as np
import ml_dtypes

import concourse.bacc as bacc
import concourse.mybir as mybir
import concourse.tile as tile
from concourse.bass_utils import run_bass_kernel_spmd

BF16 = mybir.dt.bfloat16
I16 = mybir.dt.int16
F32 = mybir.dt.float32
AF = mybir.ActivationFunctionType
ALU = mybir.AluOpType

B, S, D = 2, 2048, 1024
H = 16
DK = 64
NCORES = 8
HG = 4  # head groups
HPG = 4  # heads per group
GO = HPG * DK  # 256 group output width
NIC = D // 128  # 8 contraction chunks
NSC = S // 128  # 16 sk chunks
NSQ = S // 512  # 4 sq chunks

# Schraudolph bf16 exp: bf16 bits = round(x*scale*128/ln2 + 127*128 - 8)
SCHRAUD_MUL = 0.125 * 128.0 / math.log(2.0)
SCHRAUD_ADD = 127.0 * 128.0 - 8.0

_NC = None


def _emit(nc, tc, io):
    outT = io["outT"]

    with (
        tc.tile_pool(name="wp", bufs=1) as wp,
        tc.tile_pool(name="xp", bufs=1) as xp,
        tc.tile_pool(name="pp", bufs=1) as pp,
        tc.tile_pool(name="pt", bufs=6) as ptp,
        tc.tile_pool(name="rr", bufs=2) as rrp,
        tc.tile_pool(name="rb", bufs=6) as rbp,
        tc.tile_pool(name="fo", bufs=8) as fop,
        tc.tile_pool(name="psS", bufs=2, space="PSUM") as psS,
        tc.tile_pool(name="psPV", bufs=2, space="PSUM") as psPV,
        tc.tile_pool(name="psRS", bufs=1, space="PSUM") as psRS,
        tc.tile_pool(name="psX", bufs=1, space="PSUM") as psX,
    ):
        # ---------------- tiles ----------------
        bq2 = wp.tile([128, 2], F32, name="bq2", tag="bq2")
        wkM = wp.tile([128, NIC * GO], BF16, name="wkM", tag="wkM")
        wqM = wp.tile([128, NIC * GO], BF16, name="wqM", tag="wqM")
        wvM = wp.tile([128, NIC * GO], BF16, name="wvM", tag="wvM")
        woM = wp.tile([128, 2 * D], BF16, name="woM", tag="woM")
        wk = [wkM[:, GO * i:GO * i + GO] for i in range(NIC)]
        wq = [wqM[:, GO * i:GO * i + GO] for i in range(NIC)]
        wv = [wvM[:, GO * i:GO * i + GO] for i in range(NIC)]
        wo = [woM[:, D * o:D * o + D] for o in range(2)]
        ones_col = wp.tile([128, 32], BF16, name="ones", tag="ones")
        # selector: sel[32h, 64h:64h+64] = 1 — K=128 matmul against it
        # broadcasts partition 32h across 64 output partitions
        sel = wp.tile([128, 4 * 64], BF16, name="sel", tag="sel")

        # packed X: one [128, 4096] tile per s-column chunk c (col =
        # 512*ic within a chunk). One tile per DMA keeps read-after-write
        # dependencies exact — a merged mega-tile made every matmul wait on
        # ALL its DMA writers.
        xkC = [xp.tile([128, NIC * 512], BF16, name=f"xk{c}", tag=f"xk{c}")
               for c in range(4)]
        xqC = [xp.tile([128, NIC * 512], BF16, name=f"xq{c}", tag=f"xq{c}")
               for c in range(4)]
        xvC = [xp.tile([128, NIC * 512], BF16, name=f"xv{c}", tag=f"xv{c}")
               for c in range(4)]

        def xcol(xC, i, c):
            return xC[c][:, 512 * i:512 * i + 512]

        kT = [[pp.tile([128, 512], BF16, name=f"kT{o}_{c}", tag=f"kT{o}_{c}")
               for c in range(4)] for o in range(2)]
        qT = [[pp.tile([128, 512], BF16, name=f"qT{o}_{c}", tag=f"qT{o}_{c}")
               for c in range(4)] for o in range(2)]
        v = [pp.tile([128, GO], BF16, name=f"v{k}", tag=f"v{k}") for k in range(NSC)]
        attnT = [[pp.tile([128, 512], BF16, name=f"at{o}_{c}", tag=f"at{o}_{c}")
                  for c in range(4)] for o in range(2)]

        nc.vector.memset(ones_col[:], 1.0)
        nc.vector.memset(sel[:], 0.0)
        for h in range(HPG):
            nc.vector.memset(sel[32 * h:32 * h + 1, 64 * h:64 * h + 64], 1.0)

        # ---------------- DMAs ----------------
        # big c-major chunks over THREE issue queues (each sustains only
        # ~150GB/s): sync = wk + all xk (+wo, xq c3); scalar = xq c0/c1 +
        # bq2 (prefix-only, before the exp stream starts); pool = wq, wv,
        # all xv, xq c2. Ordered by first-use deadline.
        def xdma(eng, xC, name, c, half=None):
            lo, hi = (0, 4096) if half is None else (2048 * half, 2048 * half + 2048)
            eng.dma_start(xC[c][:, lo:hi], io[name][:, 4096 * c + lo:4096 * c + hi])

        # phase 1: only what the kq c0 prefix + first v-proj chunks need,
        # spread across all three issue queues (~220GB/s each, depth 2).
        # Waits coalesce per queue, so later bulk DMAs are emitted AFTER
        # the prefix compute to keep its waits tight.
        nc.sync.dma_start(wkM[:, 0:1024], io["wkP"][:, 0:1024])
        nc.sync.dma_start(wkM[:, 1024:2048], io["wkP"][:, 1024:2048])
        xdma(nc.sync, xkC, "xkP", 0, 0)
        xdma(nc.scalar, xkC, "xkP", 0, 1)
        xdma(nc.scalar, xqC, "xqP", 0, 0)
        nc.scalar.dma_start(bq2[:], io["bq2"][:])
        nc.gpsimd.dma_start(wqM[:], io["wqP"][:])
        xdma(nc.gpsimd, xqC, "xqP", 0, 1)
        nc.gpsimd.dma_start(wvM[:], io["wvP"][:])
        xdma(nc.gpsimd, xvC, "xvP", 0)

        # ---------------- projection helpers ----------------
        def kq_drain(out_kq, proj, c, oc, ps):
            if proj == "q":
                # bq folds into the drain; Identity shares act tables w/ Exp
                nc.scalar.activation(out_kq[oc][c][:], ps, AF.Identity,
                                     bias=bq2[:, oc:oc + 1])
            else:
                nc.vector.tensor_copy(out_kq[oc][c][:], ps)

        def kqproj_chunk_psS(out_kq, w, xM, proj, c):
            # prefix-only: both oc halves in one [128,1024] scores-pool tile
            ps = psS.tile([128, 1024], F32, name="s", tag="s")
            for oc in range(2):
                for ic in range(NIC):
                    nc.tensor.matmul(
                        ps[:, 512 * oc:512 * oc + 512],
                        w[ic][:, 128 * oc:128 * oc + 128],
                        xcol(xM, ic, c),
                        start=(ic == 0),
                        stop=(ic == NIC - 1),
                    )
            for oc in range(2):
                kq_drain(out_kq, proj, c, oc, ps[:, 512 * oc:512 * oc + 512])

        def kqproj_half_aux(out_kq, w, xM, proj, c, oc):
            ps = psX.tile([128, 512], F32, name="x", tag="x")
            for ic in range(NIC):
                nc.tensor.matmul(
                    ps[:],
                    w[ic][:, 128 * oc:128 * oc + 128],
                    xcol(xM, ic, c),
                    start=(ic == 0),
                    stop=(ic == NIC - 1),
                )
            kq_drain(out_kq, proj, c, oc, ps[:])

        def vproj_chunk(k):
            c, j = k // 4, k % 4
            ps = psX.tile([128, 512], F32, name="x", tag="x")
            for ic in range(NIC):
                nc.tensor.matmul(
                    ps[:, 0:GO],
                    xcol(xvC, ic, c)[:, 128 * j:128 * j + 128],
                    wv[ic],
                    start=(ic == 0),
                    stop=(ic == NIC - 1),
                )
            nc.scalar.copy(v[k][:], ps[:, 0:GO])

        def fproj_chunk(sqc, mc, fac=None):
            # out-projection chunk; drain alternates ACT/DVE, DMA sync/pool
            if fac is None:
                fac = psX.tile([128, 512], F32, name="x", tag="x")[:]
            for oc in range(2):
                nc.tensor.matmul(
                    fac,
                    wo[oc][:, 128 * mc:128 * mc + 128],
                    attnT[oc][sqc][:],
                    start=(oc == 0),
                    stop=(oc == 1),
                )
            fo_ = fop.tile([128, 512], BF16, name="fo", tag="fo")
            if mc % 2 == 0:
                nc.scalar.copy(fo_[:], fac)
            else:
                nc.vector.tensor_copy(fo_[:], fac)
            eng = nc.sync if mc % 2 == 0 else nc.gpsimd
            eng.dma_start(
                outT[128 * mc:128 * mc + 128, 512 * sqc:512 * sqc + 512],
                fo_[:],
            )

        # ---------------- attention ----------------
        def emit_scores_p(sqc, skc, p, exp_on_act=False):
            kc, kj = skc // 4, skc % 4
            ps_ = psS.tile([128, 1024], F32, name="s", tag="s")
            for sub in range(2):
                nc.tensor.matmul(
                    ps_[:, 512 * sub:512 * sub + 512],
                    kT[p][kc][64 * sub:64 * sub + 64,
                              128 * kj:128 * kj + 128],
                    qT[p][sqc][64 * sub:64 * sub + 64, :],
                    start=True,
                    stop=True,
                    tile_position=(64 * sub, 0),
                )
            pT_ = ptp.tile([128, 1024], BF16, name="pT", tag="pT")
            if p == 0 or exp_on_act:
                nc.scalar.activation(pT_[:], ps_[:], AF.Exp, scale=0.125)
            else:
                nc.vector.tensor_scalar(
                    pT_.bitcast(I16)[:], ps_[:],
                    SCHRAUD_MUL, SCHRAUD_ADD, op0=ALU.mult, op1=ALU.add)
            return pT_

        def emit_pv_rs(accP, rs, pTs, k):
            for p in range(2):
                nc.tensor.matmul(
                    accP[p][0:64, :],
                    v[k][:, 128 * p:128 * p + 64],
                    pTs[p][:, 0:512],
                    start=(k == 0),
                    stop=(k == NSC - 1),
                    tile_position=(0, 0),
                )
                nc.tensor.matmul(
                    accP[p][64:128, :],
                    v[k][:, 128 * p + 64:128 * p + 128],
                    pTs[p][:, 512:1024],
                    start=(k == 0),
                    stop=(k == NSC - 1),
                    tile_position=(0, 64),
                )
            for h in range(HPG):
                # M=32 fills every rs partition with finite sums (a later
                # selector matmul contracts ALL partitions — stale PSUM rows
                # would poison it with NaN)
                nc.tensor.matmul(
                    rs[32 * h:32 * h + 32, :],
                    ones_col[:],
                    pTs[h // 2][:, 512 * (h % 2):512 * (h % 2) + 512],
                    start=(k == 0),
                    stop=(k == NSC - 1),
                    tile_position=(0, 32 * h),
                )

        def normalize(sqc, accP, rs):
            # stage accP (frees psPV), reciprocal, then K=1 PE matmuls
            # broadcast each head's reciprocal row across its 64 dk
            # partitions into PSUM banks that sit idle across the boundary
            # (psX between proj chunks, psRS after the reciprocal) — no DMA
            # roundtrip anywhere.
            st = []
            for p in range(2):
                st_ = rbp.tile([128, 512], F32, name="st", tag="st")
                if p == 0:
                    nc.vector.tensor_copy(st_[:], accP[p][:])
                else:
                    nc.scalar.copy(st_[:], accP[p][:])
                st.append(st_)
            rr = rrp.tile([128, 512], F32, name="rr", tag="rr")
            nc.vector.reciprocal_approx_fast(rr[:], rs[:])
            rrb = rrp.tile([128, 512], BF16, name="rrb", tag="rrb")
            nc.scalar.copy(rrb[:], rr[:])
            rb_ps = [psX.tile([128, 512], F32, name="x", tag="x"),
                     psRS.tile([128, 512], F32, name="rs", tag="rs")]
            for h in range(HPG):
                p, sub = h // 2, h % 2
                nc.tensor.matmul(
                    rb_ps[p][64 * sub:64 * sub + 64, :],
                    sel[:, 64 * h:64 * h + 64],
                    rrb[:],
                    start=True,
                    stop=True,
                    tile_position=(0, 64 * sub),
                )
            for p in range(2):
                nc.vector.tensor_tensor(
                    attnT[p][sqc][:], st[p][:], rb_ps[p][:], op=ALU.mult)

        # ---------------- prefix ----------------
        kqproj_chunk_psS(kT, wk, xkC, "k", 0)
        kqproj_chunk_psS(qT, wq, xqC, "q", 0)

        # phase 2 DMAs: bulk loads, deadline-ordered
        xdma(nc.sync, xkC, "xkP", 1)
        xdma(nc.scalar, xqC, "xqP", 1)
        xdma(nc.gpsimd, xvC, "xvP", 1)
        xdma(nc.sync, xkC, "xkP", 2)
        xdma(nc.gpsimd, xvC, "xvP", 2)
        xdma(nc.sync, xkC, "xkP", 3)
        xdma(nc.gpsimd, xvC, "xvP", 3)
        nc.sync.dma_start(woM[:], io["woP"][:])
        xdma(nc.gpsimd, xqC, "xqP", 2)
        xdma(nc.sync, xqC, "xqP", 3)

        # interleave: emitted BETWEEN PV(t-2) and scores(t) so the exp
        # deadline (psS recycle) always has PE work in front of it
        KQMAP = {2: ("k", 1, 0), 3: ("k", 1, 1),
                 5: ("k", 2, 0), 6: ("k", 2, 1),
                 8: ("k", 3, 0), 9: ("k", 3, 1),
                 12: ("q", 1, 0)}

        def interleave(sqc, skc):
            # emitted between PV(t-2) and scores — must never wait on DMAs
            # that arrive later than the scores' own inputs
            if sqc == 0:
                if skc in KQMAP:
                    proj, c, oc = KQMAP[skc]
                    if proj == "k":
                        kqproj_half_aux(kT, wk, xkC, "k", c, oc)
                    else:
                        kqproj_half_aux(qT, wq, xqC, "q", c, oc)
            else:
                if sqc < 3 and skc == 12:
                    kqproj_half_aux(qT, wq, xqC, "q", sqc + 1, 0)
                if 3 <= skc <= (9 if sqc == 3 else 10):
                    fproj_chunk(sqc - 1, skc - 3)

        def interleave_post(sqc, skc):
            # emitted after the scores pairs: sweep-0 v-proj rides here so
            # the early scores never wait on the (later-arriving) xv DMAs
            if sqc == 0:
                vproj_chunk(skc)

        def boundary_fill(sqc):
            # PE work between PV(14) and PV(15) at the sweep boundary
            if sqc < 3:
                kqproj_half_aux(qT, wq, xqC, "q", sqc + 1, 1)
            else:
                fproj_chunk(2, 7)

        # ---------------- attention: lag-2 pipeline ----------------
        for sqc in range(NSQ):
            accP = [psPV.tile([128, 512], F32, name="pv", tag="pv")
                    for _ in range(2)]
            rs = psRS.tile([128, 512], F32, name="rs", tag="rs")
            pend = []
            for skc in range(NSC):
                if len(pend) == 2:
                    emit_pv_rs(accP, rs, *pend.pop(0))
                interleave(sqc, skc)
                pT0 = emit_scores_p(sqc, skc, 0)
                # first two steps after a boundary: ACT covers p1's exp too
                # (DVE is busy with the previous sweep's normalize chain)
                pT1 = emit_scores_p(sqc, skc, 1,
                                    exp_on_act=(sqc > 0 and skc < 2))
                interleave_post(sqc, skc)
                pend.append(([pT0, pT1], skc))
            emit_pv_rs(accP, rs, *pend.pop(0))
            boundary_fill(sqc)
            emit_pv_rs(accP, rs, *pend.pop(0))
            normalize(sqc, accP, rs)

        # ---------------- tail ----------------
        # final out-projection through 6 rotating PSUM slots
        s0 = psS.tile([128, 1024], F32, name="s", tag="s")
        s1 = psS.tile([128, 1024], F32, name="s", tag="s")
        slots = [s0[:, 0:512], s0[:, 512:1024], None,
                 s1[:, 0:512], s1[:, 512:1024],
                 s0[:, 0:512], s0[:, 512:1024], s1[:, 0:512]]
        for mc in range(D // 128):
            fproj_chunk(NSQ - 1, mc, fac=slots[mc])


def build_nc():
    nc = bacc.Bacc("TRN2", target_bir_lowering=False, debug=False,
                   num_devices=NCORES)
    io = {
        "xqP": nc.dram_tensor("xqP", [128, 4 * NIC * 512], BF16,
                              kind="ExternalInput").ap(),
        "xkP": nc.dram_tensor("xkP", [128, 4 * NIC * 512], BF16,
                              kind="ExternalInput").ap(),
        "xvP": nc.dram_tensor("xvP", [128, 4 * NIC * 512], BF16,
                              kind="ExternalInput").ap(),
        "bq2": nc.dram_tensor("bq2", [128, 2], F32, kind="ExternalInput").ap(),
        "wkP": nc.dram_tensor("wkP", [128, NIC * GO], BF16, kind="ExternalInput").ap(),
        "wqP": nc.dram_tensor("wqP", [128, NIC * GO], BF16, kind="ExternalInput").ap(),
        "wvP": nc.dram_tensor("wvP", [128, NIC * GO], BF16, kind="ExternalInput").ap(),
        "woP": nc.dram_tensor("woP", [128, 2 * D], BF16, kind="ExternalInput").ap(),
        "outT": nc.dram_tensor("outT", [D, S], BF16, kind="ExternalOutput").ap(),
    }
    with tile.TileContext(nc) as tc:
        _emit(nc, tc, io)
    nc.compile()
    return nc


def get_nc():
    global _NC
    if _NC is None:
        _NC = build_nc()
    return _NC


def _pack(w):
    # [n*128, m] -> [128, n*m]: partition-contiguous
    n = w.shape[0] // 128
    return np.ascontiguousarray(
        w.reshape(n, 128, w.shape[1]).transpose(1, 0, 2).reshape(128, -1)
    ).astype(ml_dtypes.bfloat16)


def _pack_x(xT):
    # [1024, 2048] -> [128, 16384] with col = 4096*c + 512*ic
    return np.ascontiguousarray(
        xT.reshape(NIC, 128, 4, 512).transpose(1, 2, 0, 3).reshape(128, -1)
    ).astype(ml_dtypes.bfloat16)


def shard_inputs(Q, K, V, Wq, bq, Wk, bk, Wv, bv, Wo, bo):
    in_maps = []
    for core in range(NCORES):
        b, hg = core // HG, core % HG
        rows = slice(GO * hg, GO * hg + GO)
        bq_g = bq[rows]
        in_maps.append({
            "xqP": _pack_x(np.ascontiguousarray(Q[b].T)),
            "xkP": _pack_x(np.ascontiguousarray(K[b].T)),
            "xvP": _pack_x(np.ascontiguousarray(V[b].T)),
            "bq2": np.stack([bq_g[0:128], bq_g[128:256]], axis=1
                            ).astype(np.float32),
            "wkP": _pack(np.ascontiguousarray(Wk[rows].T)),
            "wqP": _pack(np.ascontiguousarray(Wq[rows].T)),
            "wvP": _pack(np.ascontiguousarray(Wv[rows].T)),
            "woP": _pack(np.ascontiguousarray(Wo[:, rows].T)),
        })
    return in_maps


def kernel(**inputs):
    args = {k: np.asarray(v) for k, v in inputs.items()}
    nc = get_nc()
    in_maps = shard_inputs(
        args["Q"], args["K"], args["V"], args["Wq"], args["bq"], args["Wk"],
        args["bk"], args["Wv"], args["bv"], args["Wo"], args["bo"],
    )
    res = run_bass_kernel_spmd(nc, in_maps, list(range(NCORES)))
    out = np.zeros((B, S, D), np.float32)
    for core in range(NCORES):
        out[core // HG] += res.results[core]["outT"].astype(np.float32).T
    # bo + bv @ Wo.T (softmax rows sum to 1, so attn@(V+bv) = attn@V + bv)
    out += (args["bo"].astype(np.float32)
            + args["bv"].astype(np.float32) @ args["Wo"].astype(np.float32).T)
    return out


# revision 45
# speedup vs baseline: 1.0839x; 1.0839x over previous
"""Multi-head attention (B=2, S=2048, D=1024, H=16) on 8 Trainium2 NeuronCores.

Sharding: core = b*4 + hg  (b = batch, hg = head-group of 4 heads).

v3 — p-state-aware single-stream schedule:
  - Attention runs as 64 (sqc, skc) steps with PV/RS lagging scores by 2.
    Per-step PE order is PV/RS(t-2) | interleave | scores-p0 | scores-p1,
    which hides each exp's latency behind the NEXT step's PV + interleave
    so the psS double-buffer never stalls the PE.
  - One exp instruction per engine per step (fixed overhead ~0.3-0.4us
    per instruction dominates splitting): ACT takes pT0 (table exp),
    DVE takes pT1 (Schraudolph bf16 bit-trick; numerator and denominator
    share the approximation so softmax cancels the sawtooth).
  - X inputs arrive pre-packed [128, 16384] (c-major) so each of xq/xk/xv
    loads with 4 big DMAs — DMA issue costs ~0.7us of engine time each,
    so descriptor count matters more than ordering finesse.
  - Bias matmuls eliminated: bk drops out (softmax shift invariance),
    bv folds into a host-side constant, bq rides the q-proj drain as an
    ACT activation(Identity, bias=per-partition-column).
  - The out-projection drains straight from PSUM to DRAM via DMA (f32
    outT; host casts) — no SBUF staging, no engine drain.
  - v-proj / k-proj c1-3 / q-proj c1-3 / out-proj interleave inside the
    attention steps; two extra slots sit at each sweep boundary between
    PV(14) and PV(15).

All matmul inputs bf16 (PSUM f32). Host adds bo + bv@Wo.T and sums the
4 head-group partials per batch.
"""

import math

import numpy as np
import ml_dtypes

import concourse.bacc as bacc
import concourse.mybir as mybir
import concourse.tile as tile
from concourse.bass_utils import run_bass_kernel_spmd

BF16 = mybir.dt.bfloat16
I16 = mybir.dt.int16
F32 = mybir.dt.float32
AF = mybir.ActivationFunctionType
ALU = mybir.AluOpType

B, S, D = 2, 2048, 1024
H = 16
DK = 64
NCORES = 8
HG = 4  # head groups
HPG = 4  # heads per group
GO = HPG * DK  # 256 group output width
NIC = D // 128  # 8 contraction chunks
NSC = S // 128  # 16 sk chunks
NSQ = S // 512  # 4 sq chunks

# Schraudolph bf16 exp: bf16 bits = round(x*scale*128/ln2 + 127*128 - 8)
SCHRAUD_MUL = 0.125 * 128.0 / math.log(2.0)
SCHRAUD_ADD = 127.0 * 128.0 - 8.0

_NC = None


def _emit(nc, tc, io):
    outT = io["outT"]

    with (
        tc.tile_pool(name="wp", bufs=1) as wp,
        tc.tile_pool(name="xp", bufs=1) as xp,
        tc.tile_pool(name="pp", bufs=1) as pp,
        tc.tile_pool(name="pt", bufs=6) as ptp,
        tc.tile_pool(name="rr", bufs=2) as rrp,
        tc.tile_pool(name="rb", bufs=6) as rbp,
        tc.tile_pool(name="fo", bufs=8) as fop,
        tc.tile_pool(name="psS", bufs=2, space="PSUM") as psS,
        tc.tile_pool(name="psPV", bufs=2, space="PSUM") as psPV,
        tc.tile_pool(name="psRS", bufs=1, space="PSUM") as psRS,
        tc.tile_pool(name="psX", bufs=1, space="PSUM") as psX,
    ):
        # ---------------- tiles ----------------
        bq2 = wp.tile([128, 2], F32, name="bq2", tag="bq2")
        wkM = wp.tile([128, NIC * GO], BF16, name="wkM", tag="wkM")
        wqM = wp.tile([128, NIC * GO], BF16, name="wqM", tag="wqM")
        wvM = wp.tile([128, NIC * GO], BF16, name="wvM", tag="wvM")
        woM = wp.tile([128, 2 * D], BF16, name="woM", tag="woM")
        wk = [wkM[:, GO * i:GO * i + GO] for i in range(NIC)]
        wq = [wqM[:, GO * i:GO * i + GO] for i in range(NIC)]
        wv = [wvM[:, GO * i:GO * i + GO] for i in range(NIC)]
        wo = [woM[:, D * o:D * o + D] for o in range(2)]
        ones_col = wp.tile([128, 32], BF16, name="ones", tag="ones")
        # selector: sel[32h, 64h:64h+64] = 1 — K=128 matmul against it
        # broadcasts partition 32h across 64 output partitions
        sel = wp.tile([128, 4 * 64], BF16, name="sel", tag="sel")

        # packed X: one [128, 4096] tile per s-column chunk c (col =
        # 512*ic within a chunk). One tile per DMA keeps read-after-write
        # dependencies exact — a merged mega-tile made every matmul wait on
        # ALL its DMA writers.
        xkC = [xp.tile([128, NIC * 512], BF16, name=f"xk{c}", tag=f"xk{c}")
               for c in range(4)]
        xqC = [xp.tile([128, NIC * 512], BF16, name=f"xq{c}", tag=f"xq{c}")
               for c in range(4)]
        xvC = [xp.tile([128, NIC * 512], BF16, name=f"xv{c}", tag=f"xv{c}")
               for c in range(4)]

        def xcol(xC, i, c):
            return xC[c][:, 512 * i:512 * i + 512]

        kT = [[pp.tile([128, 512], BF16, name=f"kT{o}_{c}", tag=f"kT{o}_{c}")
               for c in range(4)] for o in range(2)]
        qT = [[pp.tile([128, 512], BF16, name=f"qT{o}_{c}", tag=f"qT{o}_{c}")
               for c in range(4)] for o in range(2)]
        v = [pp.tile([128, GO], BF16, name=f"v{k}", tag=f"v{k}") for k in range(NSC)]
        attnT = [[pp.tile([128, 512], BF16, name=f"at{o}_{c}", tag=f"at{o}_{c}")
                  for c in range(4)] for o in range(2)]

        nc.vector.memset(ones_col[:], 1.0)
        nc.vector.memset(sel[:], 0.0)
        for h in range(HPG):
            nc.vector.memset(sel[32 * h:32 * h + 1, 64 * h:64 * h + 64], 1.0)

        # ---------------- DMAs ----------------
        # big c-major chunks over THREE issue queues (each sustains only
        # ~150GB/s): sync = wk + all xk (+wo, xq c3); scalar = xq c0/c1 +
        # bq2 (prefix-only, before the exp stream starts); pool = wq, wv,
        # all xv, xq c2. Ordered by first-use deadline.
        def xdma(eng, xC, name, c, half=None):
            lo, hi = (0, 4096) if half is None else (2048 * half, 2048 * half + 2048)
            eng.dma_start(xC[c][:, lo:hi], io[name][:, 4096 * c + lo:4096 * c + hi])

        # each queue sustains only ~110GB/s — strict critical-first order:
        # k-prefix bytes lead sync+scalar, q-prefix next, then v, then bulk
        nc.sync.dma_start(wkM[:, 0:1024], io["wkP"][:, 0:1024])
        nc.sync.dma_start(wkM[:, 1024:2048], io["wkP"][:, 1024:2048])
        xdma(nc.sync, xkC, "xkP", 0, 0)
        xdma(nc.scalar, xkC, "xkP", 0, 1)
        xdma(nc.scalar, xqC, "xqP", 0, 0)
        nc.scalar.dma_start(bq2[:], io["bq2"][:])
        nc.gpsimd.dma_start(wqM[:], io["wqP"][:])
        xdma(nc.gpsimd, xqC, "xqP", 0, 1)
        nc.gpsimd.dma_start(wvM[:], io["wvP"][:])
        xdma(nc.gpsimd, xvC, "xvP", 0)
        xdma(nc.sync, xkC, "xkP", 1)
        xdma(nc.scalar, xqC, "xqP", 1)
        xdma(nc.gpsimd, xvC, "xvP", 1)
        xdma(nc.sync, xkC, "xkP", 2)
        xdma(nc.gpsimd, xvC, "xvP", 2)
        xdma(nc.sync, xkC, "xkP", 3)
        xdma(nc.gpsimd, xvC, "xvP", 3)
        nc.sync.dma_start(woM[:], io["woP"][:])
        xdma(nc.gpsimd, xqC, "xqP", 2)
        xdma(nc.sync, xqC, "xqP", 3)

        # ---------------- projection helpers ----------------
        def kq_drain(out_kq, proj, c, oc, ps):
            if proj == "q":
                # bq folds into the drain; Identity shares act tables w/ Exp
                nc.scalar.activation(out_kq[oc][c][:], ps, AF.Identity,
                                     bias=bq2[:, oc:oc + 1])
            else:
                nc.vector.tensor_copy(out_kq[oc][c][:], ps)

        def kqproj_chunk_psS(out_kq, w, xM, proj, c):
            # prefix-only: both oc halves in one [128,1024] scores-pool tile
            ps = psS.tile([128, 1024], F32, name="s", tag="s")
            for oc in range(2):
                for ic in range(NIC):
                    nc.tensor.matmul(
                        ps[:, 512 * oc:512 * oc + 512],
                        w[ic][:, 128 * oc:128 * oc + 128],
                        xcol(xM, ic, c),
                        start=(ic == 0),
                        stop=(ic == NIC - 1),
                    )
            for oc in range(2):
                kq_drain(out_kq, proj, c, oc, ps[:, 512 * oc:512 * oc + 512])

        def kqproj_half_aux(out_kq, w, xM, proj, c, oc):
            ps = psX.tile([128, 512], F32, name="x", tag="x")
            for ic in range(NIC):
                nc.tensor.matmul(
                    ps[:],
                    w[ic][:, 128 * oc:128 * oc + 128],
                    xcol(xM, ic, c),
                    start=(ic == 0),
                    stop=(ic == NIC - 1),
                )
            kq_drain(out_kq, proj, c, oc, ps[:])

        def vproj_chunk(k):
            c, j = k // 4, k % 4
            ps = psX.tile([128, 512], F32, name="x", tag="x")
            for ic in range(NIC):
                nc.tensor.matmul(
                    ps[:, 0:GO],
                    xcol(xvC, ic, c)[:, 128 * j:128 * j + 128],
                    wv[ic],
                    start=(ic == 0),
                    stop=(ic == NIC - 1),
                )
            nc.scalar.copy(v[k][:], ps[:, 0:GO])

        def fproj_chunk(sqc, mc, fac=None):
            # out-projection chunk; drain alternates ACT/DVE, DMA sync/pool
            if fac is None:
                fac = psX.tile([128, 512], F32, name="x", tag="x")[:]
            for oc in range(2):
                nc.tensor.matmul(
                    fac,
                    wo[oc][:, 128 * mc:128 * mc + 128],
                    attnT[oc][sqc][:],
                    start=(oc == 0),
                    stop=(oc == 1),
                )
            fo_ = fop.tile([128, 512], BF16, name="fo", tag="fo")
            if mc % 2 == 0:
                nc.scalar.copy(fo_[:], fac)
            else:
                nc.vector.tensor_copy(fo_[:], fac)
            eng = nc.sync if mc % 2 == 0 else nc.gpsimd
            eng.dma_start(
                outT[128 * mc:128 * mc + 128, 512 * sqc:512 * sqc + 512],
                fo_[:],
            )

        # ---------------- attention ----------------
        def emit_scores_p(sqc, skc, p, exp_on_act=False):
            kc, kj = skc // 4, skc % 4
            ps_ = psS.tile([128, 1024], F32, name="s", tag="s")
            for sub in range(2):
                nc.tensor.matmul(
                    ps_[:, 512 * sub:512 * sub + 512],
                    kT[p][kc][64 * sub:64 * sub + 64,
                              128 * kj:128 * kj + 128],
                    qT[p][sqc][64 * sub:64 * sub + 64, :],
                    start=True,
                    stop=True,
                    tile_position=(64 * sub, 0),
                )
            pT_ = ptp.tile([128, 1024], BF16, name="pT", tag="pT")
            if p == 0 or exp_on_act:
                nc.scalar.activation(pT_[:], ps_[:], AF.Exp, scale=0.125)
            else:
                nc.vector.tensor_scalar(
                    pT_.bitcast(I16)[:], ps_[:],
                    SCHRAUD_MUL, SCHRAUD_ADD, op0=ALU.mult, op1=ALU.add)
            return pT_

        def emit_pv_rs(accP, rs, pTs, k):
            for p in range(2):
                nc.tensor.matmul(
                    accP[p][0:64, :],
                    v[k][:, 128 * p:128 * p + 64],
                    pTs[p][:, 0:512],
                    start=(k == 0),
                    stop=(k == NSC - 1),
                    tile_position=(0, 0),
                )
                nc.tensor.matmul(
                    accP[p][64:128, :],
                    v[k][:, 128 * p + 64:128 * p + 128],
                    pTs[p][:, 512:1024],
                    start=(k == 0),
                    stop=(k == NSC - 1),
                    tile_position=(0, 64),
                )
            for h in range(HPG):
                # M=32 fills every rs partition with finite sums (a later
                # selector matmul contracts ALL partitions — stale PSUM rows
                # would poison it with NaN)
                nc.tensor.matmul(
                    rs[32 * h:32 * h + 32, :],
                    ones_col[:],
                    pTs[h // 2][:, 512 * (h % 2):512 * (h % 2) + 512],
                    start=(k == 0),
                    stop=(k == NSC - 1),
                    tile_position=(0, 32 * h),
                )

        def normalize(sqc, accP, rs):
            # stage accP (frees psPV), reciprocal, then K=1 PE matmuls
            # broadcast each head's reciprocal row across its 64 dk
            # partitions into PSUM banks that sit idle across the boundary
            # (psX between proj chunks, psRS after the reciprocal) — no DMA
            # roundtrip anywhere.
            st = []
            for p in range(2):
                st_ = rbp.tile([128, 512], F32, name="st", tag="st")
                if p == 0:
                    nc.vector.tensor_copy(st_[:], accP[p][:])
                else:
                    nc.scalar.copy(st_[:], accP[p][:])
                st.append(st_)
            rr = rrp.tile([128, 512], F32, name="rr", tag="rr")
            nc.vector.reciprocal_approx_fast(rr[:], rs[:])
            rrb = rrp.tile([128, 512], BF16, name="rrb", tag="rrb")
            nc.scalar.copy(rrb[:], rr[:])
            rb_ps = [psX.tile([128, 512], F32, name="x", tag="x"),
                     psRS.tile([128, 512], F32, name="rs", tag="rs")]
            for h in range(HPG):
                p, sub = h // 2, h % 2
                nc.tensor.matmul(
                    rb_ps[p][64 * sub:64 * sub + 64, :],
                    sel[:, 64 * h:64 * h + 64],
                    rrb[:],
                    start=True,
                    stop=True,
                    tile_position=(0, 64 * sub),
                )
            for p in range(2):
                nc.vector.tensor_tensor(
                    attnT[p][sqc][:], st[p][:], rb_ps[p][:], op=ALU.mult)

        # ---------------- prefix ----------------
        kqproj_chunk_psS(kT, wk, xkC, "k", 0)
        kqproj_chunk_psS(qT, wq, xqC, "q", 0)

        # interleave: emitted BETWEEN PV(t-2) and scores(t) so the exp
        # deadline (psS recycle) always has PE work in front of it
        KQMAP = {2: ("k", 1, 0), 3: ("k", 1, 1),
                 5: ("k", 2, 0), 6: ("k", 2, 1),
                 8: ("k", 3, 0), 9: ("k", 3, 1),
                 12: ("q", 1, 0)}

        def interleave(sqc, skc):
            # emitted between PV(t-2) and scores — must never wait on DMAs
            # that arrive later than the scores' own inputs
            if sqc == 0:
                if skc in KQMAP:
                    proj, c, oc = KQMAP[skc]
                    if proj == "k":
                        kqproj_half_aux(kT, wk, xkC, "k", c, oc)
                    else:
                        kqproj_half_aux(qT, wq, xqC, "q", c, oc)
            else:
                if sqc < 3 and skc == 12:
                    kqproj_half_aux(qT, wq, xqC, "q", sqc + 1, 0)
                if 3 <= skc <= (9 if sqc == 3 else 10):
                    fproj_chunk(sqc - 1, skc - 3)

        def interleave_post(sqc, skc):
            # emitted after the scores pairs: sweep-0 v-proj rides here so
            # the early scores never wait on the (later-arriving) xv DMAs
            if sqc == 0:
                vproj_chunk(skc)

        def boundary_fill(sqc):
            # PE work between PV(14) and PV(15) at the sweep boundary
            if sqc < 3:
                kqproj_half_aux(qT, wq, xqC, "q", sqc + 1, 1)
            else:
                fproj_chunk(2, 7)

        # ---------------- attention: lag-2 pipeline ----------------
        for sqc in range(NSQ):
            accP = [psPV.tile([128, 512], F32, name="pv", tag="pv")
                    for _ in range(2)]
            rs = psRS.tile([128, 512], F32, name="rs", tag="rs")
            pend = []
            for skc in range(NSC):
                if len(pend) == 2:
                    emit_pv_rs(accP, rs, *pend.pop(0))
                interleave(sqc, skc)
                pT0 = emit_scores_p(sqc, skc, 0)
                # first two steps after a boundary: ACT covers p1's exp too
                # (DVE is busy with the previous sweep's normalize chain)
                pT1 = emit_scores_p(sqc, skc, 1,
                                    exp_on_act=(sqc > 0 and skc < 2))
                interleave_post(sqc, skc)
                pend.append(([pT0, pT1], skc))
            emit_pv_rs(accP, rs, *pend.pop(0))
            boundary_fill(sqc)
            emit_pv_rs(accP, rs, *pend.pop(0))
            normalize(sqc, accP, rs)

        # ---------------- tail ----------------
        # final out-projection through 6 rotating PSUM slots
        s0 = psS.tile([128, 1024], F32, name="s", tag="s")
        s1 = psS.tile([128, 1024], F32, name="s", tag="s")
        slots = [s0[:, 0:512], s0[:, 512:1024], None,
                 s1[:, 0:512], s1[:, 512:1024],
                 s0[:, 0:512], s0[:, 512:1024], s1[:, 0:512]]
        for mc in range(D // 128):
            fproj_chunk(NSQ - 1, mc, fac=slots[mc])


def build_nc():
    nc = bacc.Bacc("TRN2", target_bir_lowering=False, debug=False,
                   num_devices=NCORES)
    io = {
        "xqP": nc.dram_tensor("xqP", [128, 4 * NIC * 512], BF16,
                              kind="ExternalInput").ap(),
        "xkP": nc.dram_tensor("xkP", [128, 4 * NIC * 512], BF16,
                              kind="ExternalInput").ap(),
        "xvP": nc.dram_tensor("xvP", [128, 4 * NIC * 512], BF16,
                              kind="ExternalInput").ap(),
        "bq2": nc.dram_tensor("bq2", [128, 2], F32, kind="ExternalInput").ap(),
        "wkP": nc.dram_tensor("wkP", [128, NIC * GO], BF16, kind="ExternalInput").ap(),
        "wqP": nc.dram_tensor("wqP", [128, NIC * GO], BF16, kind="ExternalInput").ap(),
        "wvP": nc.dram_tensor("wvP", [128, NIC * GO], BF16, kind="ExternalInput").ap(),
        "woP": nc.dram_tensor("woP", [128, 2 * D], BF16, kind="ExternalInput").ap(),
        "outT": nc.dram_tensor("outT", [D, S], BF16, kind="ExternalOutput").ap(),
    }
    with tile.TileContext(nc) as tc:
        _emit(nc, tc, io)
    nc.compile()
    return nc


def get_nc():
    global _NC
    if _NC is None:
        _NC = build_nc()
    return _NC


def _pack(w):
    # [n*128, m] -> [128, n*m]: partition-contiguous
    n = w.shape[0] // 128
    return np.ascontiguousarray(
        w.reshape(n, 128, w.shape[1]).transpose(1, 0, 2).reshape(128, -1)
    ).astype(ml_dtypes.bfloat16)


def _pack_x(xT):
    # [1024, 2048] -> [128, 16384] with col = 4096*c + 512*ic
    return np.ascontiguousarray(
        xT.reshape(NIC, 128, 4, 512).transpose(1, 2, 0, 3).reshape(128, -1)
    ).astype(ml_dtypes.bfloat16)


def shard_inputs(Q, K, V, Wq, bq, Wk, bk, Wv, bv, Wo, bo):
    in_maps = []
    for core in range(NCORES):
        b, hg = core // HG, core % HG
        rows = slice(GO * hg, GO * hg + GO)
        bq_g = bq[rows]
        in_maps.append({
            "xqP": _pack_x(np.ascontiguousarray(Q[b].T)),
            "xkP": _pack_x(np.ascontiguousarray(K[b].T)),
            "xvP": _pack_x(np.ascontiguousarray(V[b].T)),
            "bq2": np.stack([bq_g[0:128], bq_g[128:256]], axis=1
                            ).astype(np.float32),
            "wkP": _pack(np.ascontiguousarray(Wk[rows].T)),
            "wqP": _pack(np.ascontiguousarray(Wq[rows].T)),
            "wvP": _pack(np.ascontiguousarray(Wv[rows].T)),
            "woP": _pack(np.ascontiguousarray(Wo[:, rows].T)),
        })
    return in_maps


def kernel(**inputs):
    args = {k: np.asarray(v) for k, v in inputs.items()}
    nc = get_nc()
    in_maps = shard_inputs(
        args["Q"], args["K"], args["V"], args["Wq"], args["bq"], args["Wk"],
        args["bk"], args["Wv"], args["bv"], args["Wo"], args["bo"],
    )
    res = run_bass_kernel_spmd(nc, in_maps, list(range(NCORES)))
    out = np.zeros((B, S, D), np.float32)
    for core in range(NCORES):
        out[core // HG] += res.results[core]["outT"].astype(np.float32).T
    # bo + bv @ Wo.T (softmax rows sum to 1, so attn@(V+bv) = attn@V + bv)
    out += (args["bo"].astype(np.float32)
            + args["bv"].astype(np.float32) @ args["Wo"].astype(np.float32).T)
    return out
